# revision 11
# baseline (speedup 1.0000x reference)
"""Chamfer kernel v2: compute each batch's distance matrix ONCE.

8 cores = 4 batches x 2 m-halves. Core (b,h) computes D[m,n] = d2 for its
2048 predict-rows m and all 4096 gt-cols n (p2 and g2 folded into the
K-stacked bf16 matmul, so PSUM holds full d2).

  fwd: rowmin over n on the DVE (free-axis tensor_reduce from PSUM),
       complete on-core -> [128, 16] per core.
  bwd: colmin over m, partial per core, split by n-range:
       - n in [0, 2048): ScalarE copies PSUM->SBUF with scale=-1;
         GpSimd partition_all_reduce(max) gives -min over the 128 rows;
         per-m-tile partials stacked into partitions and reduced once
         more -> [1, 2048] per core (negated colmin partial).
       - n in [2048, 4096): separate "E" matmuls with roles swapped
         (gt as rows, predict as cols) -> DVE rowmin over m -> [128, 8].
Host combines partials across the 2 halves, adds nothing (d2 complete),
sqrt + means.
"""

import numpy as np
import ml_dtypes

B = 4
M = 4096
HALF = 2048
P = 128
K = 32
NTM = HALF // P          # 16 m-tiles (D side)
Q = 1920                 # n in [0, Q) handled by gpsimd partition-reduce
NTE = (M - Q) // P       # 17 n-tiles on the E side, n in [Q, 4096)
EPS = 1e-8

_PROGRAM = None


def _split3(x):
    h = x.astype(ml_dtypes.bfloat16)
    r = x - h.astype(np.float32)
    m = r.astype(ml_dtypes.bfloat16)
    r2 = r - m.astype(np.float32)
    lo = r2.astype(ml_dtypes.bfloat16)
    return [h, m, lo]


def _build_wv_full(X, Y, x2, y2):
    """Operands so PSUM = x2[m] + y2[n] - 2 x_m.y_n (full d2).

    X: (3, Mw) row points, Y: (3, Nv) col points. Returns w [K, Mw],
    v [K, Nv]."""
    Mw = X.shape[1]
    Nv = Y.shape[1]
    a = (-2.0 * X).astype(np.float32)
    asp = _split3(a)
    ysp = _split3(Y.astype(np.float32))
    y2sp = _split3(y2.astype(np.float32))
    x2sp = _split3(x2.astype(np.float32))
    w = np.zeros((K, Mw), dtype=ml_dtypes.bfloat16)
    v = np.zeros((K, Nv), dtype=ml_dtypes.bfloat16)
    r0 = 0
    for i in range(3):
        for j in range(3):
            if i == 2 and j == 2:
                continue  # a_l*y_l ~ 2^-36, negligible
            w[r0:r0 + 3] = asp[i]
            v[r0:r0 + 3] = ysp[j]
            r0 += 3
    # + y2[n]: ones weights x y2 splits
    for j in range(3):
        w[r0] = np.ones(Mw, dtype=ml_dtypes.bfloat16)
        v[r0] = y2sp[j]
        r0 += 1
    # + x2[m]: x2 splits x ones
    for j in range(3):
        w[r0] = x2sp[j]
        v[r0] = np.ones(Nv, dtype=ml_dtypes.bfloat16)
        r0 += 1
    assert r0 == 30
    return w, v


def _build_program():
    import concourse.bass as bass
    import concourse.mybir as mybir
    import concourse.bass_isa as bass_isa
    import concourse.tile as tile
    from concourse import bacc

    f32 = mybir.dt.float32
    bf16 = mybir.dt.bfloat16

    nc = bacc.Bacc()
    # D side: w [K, 2048] (core's m rows), v [K, 4096] (all n)
    w_d = nc.declare_dram_parameter("w", [K, HALF], bf16, isOutput=False)
    v_d = nc.declare_dram_parameter("v", [K, M], bf16, isOutput=False)
    # E side: rows = gt n in [Q, 4096), cols = core's m rows
    we_d = nc.declare_dram_parameter("we", [K, NTE * P], bf16, isOutput=False)
    ve_d = nc.declare_dram_parameter("ve", [K, HALF], bf16, isOutput=False)
    of_d = nc.declare_dram_parameter("of", [P, NTM], f32, isOutput=True)   # fwd rowmin
    oe_d = nc.declare_dram_parameter("oe", [P, NTE], f32, isOutput=True)   # bwd E-side rowmin
    og_d = nc.declare_dram_parameter("og", [1, Q], f32, isOutput=True)  # bwd gpsimd side (negated max)

    with tile.TileContext(nc) as tc:
        with (
            tc.tile_pool(name="inp", bufs=1) as inp_pool,
            tc.tile_pool(name="work", bufs=6) as work_pool,
            tc.tile_pool(name="acc", bufs=1) as acc_pool,
            tc.tile_pool(name="ps", bufs=2, space=bass.MemorySpace.PSUM) as ps_pool,
        ):
            w_s = inp_pool.tile([K, HALF], bf16)
            v_s = inp_pool.tile([K, M], bf16)
            we_s = inp_pool.tile([K, NTE * P], bf16)
            ve_s = inp_pool.tile([K, HALF], bf16)
            nc.sync.dma_start(w_s[:, 0:512], w_d[:, 0:512])
            for c in range(4):
                nc.sync.dma_start(v_s[:, c * 1024:(c + 1) * 1024],
                                  v_d[:, c * 1024:(c + 1) * 1024])
            for c in range(1, 4):
                nc.sync.dma_start(w_s[:, c * 512:(c + 1) * 512],
                                  w_d[:, c * 512:(c + 1) * 512])
            nc.sync.dma_start(we_s[:], we_d[:])
            nc.sync.dma_start(ve_s[:], ve_d[:])

            of_sb = acc_pool.tile([P, NTM], f32)
            oe_sb = acc_pool.tile([P, NTE], f32)
            partf = acc_pool.tile([P, NTM, 2], f32)
            comb = acc_pool.tile([NTM, Q], f32)  # stacked gpsimd partials
            comb_out = acc_pool.tile([NTM, Q], f32)

            # Interleave: per m-tile iteration, the two D-chunks (s0 feeds
            # gpsimd bwd partial + DVE fwd rowmin, s1 DVE only) plus ~1
            # E-side chunk, so the DVE's E work overlaps the gpsimd-bound
            # D phase instead of serializing at the end.
            parte = acc_pool.tile([P, NTE], f32)
            e_done = 0
            for mt in range(NTM):
                wt = w_s[:, mt * P:(mt + 1) * P]
                for s in range(2):
                    n0 = s * 2048
                    ps = ps_pool.tile([P, 2048], f32, tag="ps")
                    for j in range(4):
                        nc.tensor.matmul(ps[:, j * 512:(j + 1) * 512], wt,
                                         v_s[:, n0 + j * 512:n0 + (j + 1) * 512])
                    # fwd rowmin on DVE
                    nc.vector.tensor_reduce(partf[:, mt, s:s + 1], ps[:],
                                            axis=mybir.AxisListType.X,
                                            op=mybir.AluOpType.min)
                    if s == 0:
                        # bwd gpsimd share: negate-copy + partition max
                        cp = work_pool.tile([P, Q], f32, tag="cp")
                        nc.scalar.mul(cp[:], ps[:, 0:Q], -1.0)
                        par = work_pool.tile([P, Q], f32, tag="par")
                        nc.gpsimd.partition_all_reduce(par[:], cp[:], P,
                                                       bass_isa.ReduceOp.max)
                        nc.sync.dma_start(comb[mt:mt + 1, :], par[0:1, :])

                n_e = 2 if mt == 0 else 1
                for _ in range(n_e):
                    if e_done >= NTE:
                        continue
                    nt = e_done
                    wte = we_s[:, nt * P:(nt + 1) * P]
                    pse = ps_pool.tile([P, 2048], f32, tag="ps")
                    for j in range(4):
                        nc.tensor.matmul(pse[:, j * 512:(j + 1) * 512], wte,
                                         ve_s[:, j * 512:(j + 1) * 512])
                    nc.vector.tensor_reduce(parte[:, nt:nt + 1], pse[:],
                                            axis=mybir.AxisListType.X,
                                            op=mybir.AluOpType.min)
                    e_done += 1

            nc.vector.tensor_reduce(of_sb[:], partf[:],
                                    axis=mybir.AxisListType.X,
                                    op=mybir.AluOpType.min)
            nc.sync.dma_start(of_d[:], of_sb[:])

            # combine the 16 stacked gpsimd partials
            nc.gpsimd.partition_all_reduce(comb_out[:], comb[:], NTM,
                                           bass_isa.ReduceOp.max)
            nc.sync.dma_start(og_d[:], comb_out[0:1, :])

            nc.vector.tensor_copy(oe_sb[:], parte[:])
            nc.sync.dma_start(oe_d[:], oe_sb[:])

    if not nc.is_finalized():
        nc.finalize()
    return nc


def _make_in_maps(p, g):
    p2 = np.sum(p * p, axis=1, dtype=np.float32)  # (B, M)
    g2 = np.sum(g * g, axis=1, dtype=np.float32)  # (B, N)
    in_maps = []
    for b in range(B):
        for h in range(2):
            sl = slice(h * HALF, (h + 1) * HALF)
            w, v = _build_wv_full(p[b][:, sl], g[b], p2[b][sl], g2[b])
            we, ve = _build_wv_full(g[b][:, Q:], p[b][:, sl],
                                    g2[b][Q:], p2[b][sl])
            in_maps.append({"w": w, "v": v, "we": we, "ve": ve})
    return in_maps


def kernel(predict_pc, gt_pc):
    from concourse.bass_utils import run_bass_kernel_spmd

    global _PROGRAM
    if _PROGRAM is None:
        _PROGRAM = _build_program()
    nc = _PROGRAM

    p = np.asarray(predict_pc, dtype=np.float32)
    g = np.asarray(gt_pc, dtype=np.float32)

    in_maps = _make_in_maps(p, g)
    res = run_bass_kernel_spmd(nc, in_maps, core_ids=list(range(8)))

    fwd_elems = []
    bwd_min2 = np.full((B, M), np.inf)
    for i in range(2 * B):
        b, h = divmod(i, 2)
        r = res.results[i]
        fwd = np.asarray(r["of"], dtype=np.float64).T.reshape(HALF)
        fwd_elems.append(fwd)
        # gpsimd side: negated colmin partial for n in [0, Q)
        gp = -np.asarray(r["og"], dtype=np.float64).reshape(Q)
        bwd_min2[b, :Q] = np.minimum(bwd_min2[b, :Q], gp)
        # E side: colmin partial for n in [Q, 4096)
        ee = np.asarray(r["oe"], dtype=np.float64).T.reshape(M - Q)
        bwd_min2[b, Q:] = np.minimum(bwd_min2[b, Q:], ee)

    fwd_min2 = np.concatenate(fwd_elems)  # B*M values (order: b0h0, b0h1, ...)
    fwd_mean = np.sqrt(np.maximum(fwd_min2, 0.0) + EPS).mean()
    bwd_mean = np.sqrt(np.maximum(bwd_min2.reshape(-1), 0.0) + EPS).mean()
    return np.array(fwd_mean + bwd_mean, dtype=np.float32)


# revision 12
# speedup vs baseline: 1.1493x; 1.1493x over previous
"""Chamfer kernel v2: compute each batch's distance matrix ONCE.

8 cores = 4 batches x 2 m-halves. Core (b,h) computes D[m,n] = d2 for its
2048 predict-rows m and all 4096 gt-cols n (p2 and g2 folded into the
K-stacked bf16 matmul, so PSUM holds full d2).

  fwd: rowmin over n on the DVE (free-axis tensor_reduce from PSUM),
       complete on-core -> [128, 16] per core.
  bwd: colmin over m, partial per core, split by n-range:
       - n in [0, 2048): ScalarE copies PSUM->SBUF with scale=-1;
         GpSimd partition_all_reduce(max) gives -min over the 128 rows;
         per-m-tile partials stacked into partitions and reduced once
         more -> [1, 2048] per core (negated colmin partial).
       - n in [2048, 4096): separate "E" matmuls with roles swapped
         (gt as rows, predict as cols) -> DVE rowmin over m -> [128, 8].
Host combines partials across the 2 halves, adds nothing (d2 complete),
sqrt + means.
"""

import numpy as np
import ml_dtypes

B = 4
M = 4096
HALF = 2048
P = 128
K = 32
NTM = HALF // P          # 16 m-tiles (D side)
Q = 1920                 # n in [0, Q) handled by gpsimd partition-reduce
NTE = (M - Q) // P       # 17 n-tiles on the E side, n in [Q, 4096)
EPS = 1e-8

_PROGRAM = None


def _split3(x):
    h = x.astype(ml_dtypes.bfloat16)
    r = x - h.astype(np.float32)
    m = r.astype(ml_dtypes.bfloat16)
    r2 = r - m.astype(np.float32)
    lo = r2.astype(ml_dtypes.bfloat16)
    return [h, m, lo]


def _build_wv_full(X, Y, x2, y2):
    """Operands so PSUM = x2[m] + y2[n] - 2 x_m.y_n (full d2).

    X: (3, Mw) row points, Y: (3, Nv) col points. Returns w [K, Mw],
    v [K, Nv]."""
    Mw = X.shape[1]
    Nv = Y.shape[1]
    a = (-2.0 * X).astype(np.float32)
    asp = _split3(a)
    ysp = _split3(Y.astype(np.float32))
    y2sp = _split3(y2.astype(np.float32))
    x2sp = _split3(x2.astype(np.float32))
    w = np.zeros((K, Mw), dtype=ml_dtypes.bfloat16)
    v = np.zeros((K, Nv), dtype=ml_dtypes.bfloat16)
    r0 = 0
    for i in range(3):
        for j in range(3):
            if i == 2 and j == 2:
                continue  # a_l*y_l ~ 2^-36, negligible
            w[r0:r0 + 3] = asp[i]
            v[r0:r0 + 3] = ysp[j]
            r0 += 3
    # + y2[n]: ones weights x y2 splits
    for j in range(3):
        w[r0] = np.ones(Mw, dtype=ml_dtypes.bfloat16)
        v[r0] = y2sp[j]
        r0 += 1
    # + x2[m]: x2 splits x ones
    for j in range(3):
        w[r0] = x2sp[j]
        v[r0] = np.ones(Nv, dtype=ml_dtypes.bfloat16)
        r0 += 1
    assert r0 == 30
    return w, v


def _build_program():
    import concourse.bass as bass
    import concourse.mybir as mybir
    import concourse.bass_isa as bass_isa
    import concourse.tile as tile
    from concourse import bacc

    f32 = mybir.dt.float32
    bf16 = mybir.dt.bfloat16

    nc = bacc.Bacc()
    # D side: w [K, 2048] (core's m rows), v [K, 4096] (all n)
    w_d = nc.declare_dram_parameter("w", [K, HALF], bf16, isOutput=False)
    v_d = nc.declare_dram_parameter("v", [K, M], bf16, isOutput=False)
    # E side: rows = gt n in [Q, 4096), cols = core's m rows
    we_d = nc.declare_dram_parameter("we", [K, NTE * P], bf16, isOutput=False)
    ve_d = nc.declare_dram_parameter("ve", [K, HALF], bf16, isOutput=False)
    of_d = nc.declare_dram_parameter("of", [P, NTM], f32, isOutput=True)   # fwd rowmin
    oe_d = nc.declare_dram_parameter("oe", [P, NTE], f32, isOutput=True)   # bwd E-side rowmin
    og_d = nc.declare_dram_parameter("og", [1, Q], f32, isOutput=True)  # bwd gpsimd side (negated max)

    with tile.TileContext(nc) as tc:
        with (
            tc.tile_pool(name="inp", bufs=1) as inp_pool,
            tc.tile_pool(name="work", bufs=6) as work_pool,
            tc.tile_pool(name="acc", bufs=1) as acc_pool,
            tc.tile_pool(name="ps", bufs=2, space=bass.MemorySpace.PSUM) as ps_pool,
        ):
            w_s = inp_pool.tile([K, HALF], bf16)
            v_s = inp_pool.tile([K, M], bf16)
            we_s = inp_pool.tile([K, NTE * P], bf16)
            ve_s = inp_pool.tile([K, HALF], bf16)
            nc.sync.dma_start(w_s[:, 0:512], w_d[:, 0:512])
            for c in range(4):
                nc.sync.dma_start(v_s[:, c * 1024:(c + 1) * 1024],
                                  v_d[:, c * 1024:(c + 1) * 1024])
            for c in range(1, 4):
                nc.sync.dma_start(w_s[:, c * 512:(c + 1) * 512],
                                  w_d[:, c * 512:(c + 1) * 512])
            nc.sync.dma_start(we_s[:], we_d[:])
            nc.sync.dma_start(ve_s[:], ve_d[:])

            of_sb = acc_pool.tile([P, NTM], f32)
            oe_sb = acc_pool.tile([P, NTE], f32)
            partf = acc_pool.tile([P, NTM, 4], f32)
            parte = acc_pool.tile([P, NTE, 2], f32)
            comb = acc_pool.tile([NTM, Q], f32)  # stacked gpsimd partials
            comb_out = acc_pool.tile([NTM, Q], f32)

            # D and E rings use separate PSUM tags (4 banks each) so the
            # DVE can drain E chunks during the gpsimd-bound D phase
            # without coupling the two pipelines through one buffer ring.
            # D-side: 16 m-tiles x 4 n-chunks of [128,1024].
            # E-side: NTE n-tiles x 2 m-chunks of [128,1024], interleaved.
            e_done = 0
            for mt in range(NTM):
                wt = w_s[:, mt * P:(mt + 1) * P]
                cp = work_pool.tile([P, Q], f32, tag="cp")
                for s in range(4):
                    n0 = s * 1024
                    ps = ps_pool.tile([P, 1024], f32, tag="ps")
                    for j in range(2):
                        nc.tensor.matmul(ps[:, j * 512:(j + 1) * 512], wt,
                                         v_s[:, n0 + j * 512:n0 + (j + 1) * 512])
                    # fwd rowmin on DVE
                    nc.vector.tensor_reduce(partf[:, mt, s:s + 1], ps[:],
                                            axis=mybir.AxisListType.X,
                                            op=mybir.AluOpType.min)
                    # bwd gpsimd share: negate-copy of the n < Q columns
                    if n0 < Q:
                        w_cols = min(1024, Q - n0)
                        nc.scalar.mul(cp[:, n0:n0 + w_cols],
                                      ps[:, 0:w_cols], -1.0)
                par = work_pool.tile([P, Q], f32, tag="par")
                nc.gpsimd.partition_all_reduce(par[:], cp[:], P,
                                               bass_isa.ReduceOp.max)
                nc.sync.dma_start(comb[mt:mt + 1, :], par[0:1, :])

                # interleave ~1 E n-tile per m-tile iteration
                n_e = 2 if mt == 0 else 1
                for _ in range(n_e):
                    if e_done >= NTE:
                        continue
                    nt = e_done
                    wte = we_s[:, nt * P:(nt + 1) * P]
                    for s in range(2):
                        m0 = s * 1024
                        pse = ps_pool.tile([P, 1024], f32, tag="pse")
                        for j in range(2):
                            nc.tensor.matmul(
                                pse[:, j * 512:(j + 1) * 512], wte,
                                ve_s[:, m0 + j * 512:m0 + (j + 1) * 512])
                        nc.vector.tensor_reduce(parte[:, nt, s:s + 1], pse[:],
                                                axis=mybir.AxisListType.X,
                                                op=mybir.AluOpType.min)
                    e_done += 1

            nc.vector.tensor_reduce(of_sb[:], partf[:],
                                    axis=mybir.AxisListType.X,
                                    op=mybir.AluOpType.min)
            nc.sync.dma_start(of_d[:], of_sb[:])

            # combine the 16 stacked gpsimd partials
            nc.gpsimd.partition_all_reduce(comb_out[:], comb[:], NTM,
                                           bass_isa.ReduceOp.max)
            nc.sync.dma_start(og_d[:], comb_out[0:1, :])

            nc.vector.tensor_reduce(oe_sb[:], parte[:],
                                    axis=mybir.AxisListType.X,
                                    op=mybir.AluOpType.min)
            nc.sync.dma_start(oe_d[:], oe_sb[:])

    if not nc.is_finalized():
        nc.finalize()
    return nc


def _make_in_maps(p, g):
    p2 = np.sum(p * p, axis=1, dtype=np.float32)  # (B, M)
    g2 = np.sum(g * g, axis=1, dtype=np.float32)  # (B, N)
    in_maps = []
    for b in range(B):
        for h in range(2):
            sl = slice(h * HALF, (h + 1) * HALF)
            w, v = _build_wv_full(p[b][:, sl], g[b], p2[b][sl], g2[b])
            we, ve = _build_wv_full(g[b][:, Q:], p[b][:, sl],
                                    g2[b][Q:], p2[b][sl])
            in_maps.append({"w": w, "v": v, "we": we, "ve": ve})
    return in_maps


def kernel(predict_pc, gt_pc):
    from concourse.bass_utils import run_bass_kernel_spmd

    global _PROGRAM
    if _PROGRAM is None:
        _PROGRAM = _build_program()
    nc = _PROGRAM

    p = np.asarray(predict_pc, dtype=np.float32)
    g = np.asarray(gt_pc, dtype=np.float32)

    in_maps = _make_in_maps(p, g)
    res = run_bass_kernel_spmd(nc, in_maps, core_ids=list(range(8)))

    fwd_elems = []
    bwd_min2 = np.full((B, M), np.inf)
    for i in range(2 * B):
        b, h = divmod(i, 2)
        r = res.results[i]
        fwd = np.asarray(r["of"], dtype=np.float64).T.reshape(HALF)
        fwd_elems.append(fwd)
        # gpsimd side: negated colmin partial for n in [0, Q)
        gp = -np.asarray(r["og"], dtype=np.float64).reshape(Q)
        bwd_min2[b, :Q] = np.minimum(bwd_min2[b, :Q], gp)
        # E side: colmin partial for n in [Q, 4096)
        ee = np.asarray(r["oe"], dtype=np.float64).T.reshape(M - Q)
        bwd_min2[b, Q:] = np.minimum(bwd_min2[b, Q:], ee)

    fwd_min2 = np.concatenate(fwd_elems)  # B*M values (order: b0h0, b0h1, ...)
    fwd_mean = np.sqrt(np.maximum(fwd_min2, 0.0) + EPS).mean()
    bwd_mean = np.sqrt(np.maximum(bwd_min2.reshape(-1), 0.0) + EPS).mean()
    return np.array(fwd_mean + bwd_mean, dtype=np.float32)


# revision 15
# speedup vs baseline: 1.2066x; 1.0498x over previous
"""Chamfer kernel v2: compute each batch's distance matrix ONCE.

8 cores = 4 batches x 2 m-halves. Core (b,h) computes D[m,n] = d2 for its
2048 predict-rows m and all 4096 gt-cols n (p2 and g2 folded into the
K-stacked bf16 matmul, so PSUM holds full d2).

  fwd: rowmin over n on the DVE (free-axis tensor_reduce from PSUM),
       complete on-core -> [128, 16] per core.
  bwd: colmin over m, partial per core, split by n-range:
       - n in [0, 2048): ScalarE copies PSUM->SBUF with scale=-1;
         GpSimd partition_all_reduce(max) gives -min over the 128 rows;
         per-m-tile partials stacked into partitions and reduced once
         more -> [1, 2048] per core (negated colmin partial).
       - n in [2048, 4096): separate "E" matmuls with roles swapped
         (gt as rows, predict as cols) -> DVE rowmin over m -> [128, 8].
Host combines partials across the 2 halves, adds nothing (d2 complete),
sqrt + means.
"""

import numpy as np
import ml_dtypes

B = 4
M = 4096
HALF = 2048
P = 128
K = 32
NTM = HALF // P          # 16 m-tiles (D side)
Q = 1920                 # n in [0, Q) handled by gpsimd partition-reduce
NTE = (M - Q) // P       # 17 n-tiles on the E side, n in [Q, 4096)
EPS = 1e-8

_PROGRAM = None


def _split3(x):
    h = x.astype(ml_dtypes.bfloat16)
    r = x - h.astype(np.float32)
    m = r.astype(ml_dtypes.bfloat16)
    r2 = r - m.astype(np.float32)
    lo = r2.astype(ml_dtypes.bfloat16)
    return [h, m, lo]


def _build_wv_full(X, Y, x2, y2):
    """Operands so PSUM = x2[m] + y2[n] - 2 x_m.y_n (full d2).

    X: (3, Mw) row points, Y: (3, Nv) col points. Returns w [K, Mw],
    v [K, Nv]."""
    Mw = X.shape[1]
    Nv = Y.shape[1]
    a = (-2.0 * X).astype(np.float32)
    asp = _split3(a)
    ysp = _split3(Y.astype(np.float32))
    y2sp = _split3(y2.astype(np.float32))
    x2sp = _split3(x2.astype(np.float32))
    w = np.zeros((K, Mw), dtype=ml_dtypes.bfloat16)
    v = np.zeros((K, Nv), dtype=ml_dtypes.bfloat16)
    r0 = 0
    for i in range(3):
        for j in range(3):
            if i == 2 and j == 2:
                continue  # a_l*y_l ~ 2^-36, negligible
            w[r0:r0 + 3] = asp[i]
            v[r0:r0 + 3] = ysp[j]
            r0 += 3
    # + y2[n]: ones weights x y2 splits
    for j in range(3):
        w[r0] = np.ones(Mw, dtype=ml_dtypes.bfloat16)
        v[r0] = y2sp[j]
        r0 += 1
    # + x2[m]: x2 splits x ones
    for j in range(3):
        w[r0] = x2sp[j]
        v[r0] = np.ones(Nv, dtype=ml_dtypes.bfloat16)
        r0 += 1
    assert r0 == 30
    return w, v


def _build_program():
    import concourse.bass as bass
    import concourse.mybir as mybir
    import concourse.bass_isa as bass_isa
    import concourse.tile as tile
    from concourse import bacc

    f32 = mybir.dt.float32
    bf16 = mybir.dt.bfloat16

    nc = bacc.Bacc()
    # D side: w [K, 2048] (core's m rows), v [K, 4096] (all n)
    w_d = nc.declare_dram_parameter("w", [K, HALF], bf16, isOutput=False)
    v_d = nc.declare_dram_parameter("v", [K, M], bf16, isOutput=False)
    # E side: rows = gt n in [Q, 4096), cols = core's m rows
    we_d = nc.declare_dram_parameter("we", [K, NTE * P], bf16, isOutput=False)
    ve_d = nc.declare_dram_parameter("ve", [K, HALF], bf16, isOutput=False)
    of_d = nc.declare_dram_parameter("of", [P, NTM], f32, isOutput=True)   # fwd rowmin
    oe_d = nc.declare_dram_parameter("oe", [P, NTE], f32, isOutput=True)   # bwd E-side rowmin
    og_d = nc.declare_dram_parameter("og", [NTM, Q], f32, isOutput=True)  # bwd gpsimd partials (negated max per m-tile)

    with tile.TileContext(nc) as tc:
        with (
            tc.tile_pool(name="inp", bufs=1) as inp_pool,
            tc.tile_pool(name="work", bufs=6) as work_pool,
            tc.tile_pool(name="acc", bufs=1) as acc_pool,
            tc.tile_pool(name="ps", bufs=2, space=bass.MemorySpace.PSUM) as ps_pool,
        ):
            w_s = inp_pool.tile([K, HALF], bf16)
            v_s = inp_pool.tile([K, M], bf16)
            we_s = inp_pool.tile([K, NTE * P], bf16)
            ve_s = inp_pool.tile([K, HALF], bf16)
            nc.sync.dma_start(w_s[:, 0:512], w_d[:, 0:512])
            for c in range(4):
                nc.sync.dma_start(v_s[:, c * 1024:(c + 1) * 1024],
                                  v_d[:, c * 1024:(c + 1) * 1024])
            for c in range(1, 4):
                nc.sync.dma_start(w_s[:, c * 512:(c + 1) * 512],
                                  w_d[:, c * 512:(c + 1) * 512])
            nc.sync.dma_start(we_s[:], we_d[:])
            nc.sync.dma_start(ve_s[:], ve_d[:])

            of_sb = acc_pool.tile([P, NTM], f32)
            oe_sb = acc_pool.tile([P, NTE], f32)
            partf = acc_pool.tile([P, NTM, 4], f32)
            parte = acc_pool.tile([P, NTE, 2], f32)

            # D and E rings use separate PSUM tags (4 banks each) so the
            # DVE can drain E chunks during the gpsimd-bound D phase
            # without coupling the two pipelines through one buffer ring.
            # D-side: 16 m-tiles x 4 n-chunks of [128,1024].
            # E-side: NTE n-tiles x 2 m-chunks of [128,1024], interleaved.
            e_done = 0
            for mt in range(NTM):
                wt = w_s[:, mt * P:(mt + 1) * P]
                cp = work_pool.tile([P, Q], f32, tag="cp")
                for s in range(4):
                    n0 = s * 1024
                    ps = ps_pool.tile([P, 1024], f32, tag="ps")
                    for j in range(2):
                        nc.tensor.matmul(ps[:, j * 512:(j + 1) * 512], wt,
                                         v_s[:, n0 + j * 512:n0 + (j + 1) * 512])
                    # fwd rowmin on DVE
                    nc.vector.tensor_reduce(partf[:, mt, s:s + 1], ps[:],
                                            axis=mybir.AxisListType.X,
                                            op=mybir.AluOpType.min)
                    # bwd gpsimd share: negate-copy of the n < Q columns
                    if n0 < Q:
                        w_cols = min(1024, Q - n0)
                        nc.scalar.mul(cp[:, n0:n0 + w_cols],
                                      ps[:, 0:w_cols], -1.0)
                par = work_pool.tile([P, Q], f32, tag="par")
                nc.gpsimd.partition_all_reduce(par[:], cp[:], P,
                                               bass_isa.ReduceOp.max)
                # partial straight to DRAM; host does the 16-way combine
                nc.sync.dma_start(og_d[mt:mt + 1, :], par[0:1, :])

                # interleave ~1 E n-tile per m-tile iteration
                n_e = 2 if mt == 0 else 1
                for _ in range(n_e):
                    if e_done >= NTE:
                        continue
                    nt = e_done
                    wte = we_s[:, nt * P:(nt + 1) * P]
                    for s in range(2):
                        m0 = s * 1024
                        pse = ps_pool.tile([P, 1024], f32, tag="pse")
                        for j in range(2):
                            nc.tensor.matmul(
                                pse[:, j * 512:(j + 1) * 512], wte,
                                ve_s[:, m0 + j * 512:m0 + (j + 1) * 512])
                        nc.vector.tensor_reduce(parte[:, nt, s:s + 1], pse[:],
                                                axis=mybir.AxisListType.X,
                                                op=mybir.AluOpType.min)
                    e_done += 1

            nc.vector.tensor_reduce(of_sb[:], partf[:],
                                    axis=mybir.AxisListType.X,
                                    op=mybir.AluOpType.min)
            nc.sync.dma_start(of_d[:], of_sb[:])

            nc.vector.tensor_reduce(oe_sb[:], parte[:],
                                    axis=mybir.AxisListType.X,
                                    op=mybir.AluOpType.min)
            nc.sync.dma_start(oe_d[:], oe_sb[:])

    if not nc.is_finalized():
        nc.finalize()
    return nc


def _make_in_maps(p, g):
    p2 = np.sum(p * p, axis=1, dtype=np.float32)  # (B, M)
    g2 = np.sum(g * g, axis=1, dtype=np.float32)  # (B, N)
    in_maps = []
    for b in range(B):
        for h in range(2):
            sl = slice(h * HALF, (h + 1) * HALF)
            w, v = _build_wv_full(p[b][:, sl], g[b], p2[b][sl], g2[b])
            we, ve = _build_wv_full(g[b][:, Q:], p[b][:, sl],
                                    g2[b][Q:], p2[b][sl])
            in_maps.append({"w": w, "v": v, "we": we, "ve": ve})
    return in_maps


def kernel(predict_pc, gt_pc):
    from concourse.bass_utils import run_bass_kernel_spmd

    global _PROGRAM
    if _PROGRAM is None:
        _PROGRAM = _build_program()
    nc = _PROGRAM

    p = np.asarray(predict_pc, dtype=np.float32)
    g = np.asarray(gt_pc, dtype=np.float32)

    in_maps = _make_in_maps(p, g)
    res = run_bass_kernel_spmd(nc, in_maps, core_ids=list(range(8)))

    fwd_elems = []
    bwd_min2 = np.full((B, M), np.inf)
    for i in range(2 * B):
        b, h = divmod(i, 2)
        r = res.results[i]
        fwd = np.asarray(r["of"], dtype=np.float64).T.reshape(HALF)
        fwd_elems.append(fwd)
        # gpsimd side: per-m-tile negated colmin partials for n in [0, Q)
        gp = -np.asarray(r["og"], dtype=np.float64).max(axis=0)
        bwd_min2[b, :Q] = np.minimum(bwd_min2[b, :Q], gp)
        # E side: colmin partial for n in [Q, 4096)
        ee = np.asarray(r["oe"], dtype=np.float64).T.reshape(M - Q)
        bwd_min2[b, Q:] = np.minimum(bwd_min2[b, Q:], ee)

    fwd_min2 = np.concatenate(fwd_elems)  # B*M values (order: b0h0, b0h1, ...)
    fwd_mean = np.sqrt(np.maximum(fwd_min2, 0.0) + EPS).mean()
    bwd_mean = np.sqrt(np.maximum(bwd_min2.reshape(-1), 0.0) + EPS).mean()
    return np.array(fwd_mean + bwd_mean, dtype=np.float32)
